# revision 1
# baseline (speedup 1.0000x reference)
"""Trainium2 Bass kernel for ContractLevelAttention (segment softmax-pooling).

Computes, for x:[N,D], sorted batch:[N] (graph ids in [0,B)), MLP weights:
    scores = tanh(x @ W1 + b1) @ W2 + b2              # [N]
    w      = segment_softmax(scores, batch)           # per-graph softmax
    out    = segment_sum(x * w[:, None], batch)       # [B, D]

Key facts exploited:
  * softmax is shift invariant and |scores| <= 1 + 128*max|W2| + |b2| ~ 11.5
    (tanh output bounded), so exp() never overflows in fp32 and the
    segment-max subtraction of the reference can be dropped entirely.
  * out[g] = (sum_i e_i x_i) / (sum_i e_i) over i in graph g, so the
    normalization happens once at the end -- both sums are plain
    segment-sums, done as one-hot matmuls on the PE.
  * the whole PE path runs in bf16 (1 cyc/row vs 4 for fp32): the host ships
    x already cast to bf16 (halves HBM traffic) and in the exact SBUF chunk
    layout; PSUM accumulation stays fp32 so segment sums are fp32-accurate.
  * PE transpose-mode runs at the cold 1.2 GHz clock, so most chunks instead
    load a host-side pre-transposed copy of x for the MLP (pure layout prep);
    1 in PE_EVERY chunks still transposes on the PE to balance DMA vs PE.

Sharding: graph-level data parallel over 8 cores (batch is sorted, so each
core's nodes are one contiguous slice, zero-padded to a fixed capacity).
"""

import numpy as np
from contextlib import ExitStack

N_FULL = 524288
D = 256
H = 128
B_FULL = 2048
NCORES = 8
B_LOC = B_FULL // NCORES      # 256 graphs per core
GCH = 128                     # graphs per PSUM accumulator chunk
NCH = B_LOC // GCH            # accumulator chunks per core
PAD_SENTINEL = 3.0 * B_LOC    # batch_rel value for padding rows (never matches)
CHT = 16                      # 128-node tiles per x DMA chunk (~2MB loads)
STT = 4                       # tiles per compute supertile
DA = D + 2                    # x cols + ones col (denominator) + pad col

_prog_cache = {}


PE_EVERY = 4  # 1 of every PE_EVERY chunks transposes on the PE; the rest
              # load the host-side pre-transposed x copy (0 = all DMA)
XLAYOUT = "sb"  # 'sb' = x shipped in SBUF chunk layout; 'row' = row-major
BUFS = {"xbp": 3, "xtp": 3, "ttp": 3, "oep": 4}
EXPSPLIT = False  # split the per-chunk exp so pooling starts earlier
XTQ = "sync"      # HWDGE queue for the xT loads: 'sync' or 'scalar'


def _build_program(C, bnds, nt_real, repeat=1, ablate="", pe_every=None,
                   xlayout=None, expsplit=None, xtq=None):
    """Build the per-core SPMD program. C = padded node capacity (multiple of
    128*CHT). bnds = tuple of (first_tile, last_tile) per graph chunk, shared
    across cores (min/max over cores). nt_real = number of tiles containing
    any real (non-padding) node. repeat>1 wraps the body in an on-device
    loop (for timing)."""
    import concourse.bass as bass
    from concourse import bacc, mybir
    import concourse.tile as tile

    if pe_every is None:
        pe_every = PE_EVERY
    if xlayout is None:
        xlayout = XLAYOUT
    if expsplit is None:
        expsplit = EXPSPLIT
    if xtq is None:
        xtq = XTQ
    f32 = mybir.dt.float32
    bf16 = mybir.dt.bfloat16
    AFT = mybir.ActivationFunctionType
    ALU = mybir.AluOpType
    T = C // 128

    nc = bacc.Bacc(
        "TRN2",
        target_bir_lowering=False,
        debug=False,
        enable_asserts=False,
        num_devices=NCORES,
    )
    # 'sb': x shipped in the exact SBUF chunk layout [128, T*DA] (host prep):
    # partition p, col block (t, d) holds x[t*128 + p, d] -> fully
    # contiguous 8.25KB-per-partition DMA descriptors per chunk.
    if xlayout == "sb":
        x_d = nc.dram_tensor("x", [128, T * DA], bf16, kind="ExternalInput").ap()
    else:
        x_d = nc.dram_tensor("x", [C, DA], bf16, kind="ExternalInput").ap()
    xt_d = nc.dram_tensor("xt", [2, 128, C], bf16, kind="ExternalInput").ap()
    brel_d = nc.dram_tensor("brel", [128, T], f32, kind="ExternalInput").ap()
    w1_d = nc.dram_tensor("w1", [2, 128, H], f32, kind="ExternalInput").ap()
    b1_d = nc.dram_tensor("b1", [H, 1], f32, kind="ExternalInput").ap()
    w2_d = nc.dram_tensor("w2", [H, 1], f32, kind="ExternalInput").ap()
    b2_d = nc.dram_tensor("b2", [128, 1], f32, kind="ExternalInput").ap()
    id_d = nc.dram_tensor("ident", [128, 128], f32, kind="ExternalInput").ap()
    iota_d = nc.dram_tensor("iota", [128, B_LOC], f32, kind="ExternalInput").ap()
    out_d = nc.dram_tensor("out", [B_LOC, D], f32, kind="ExternalOutput").ap()

    first = {c: bnds[c][0] for c in range(NCH)}
    last = {c: bnds[c][1] for c in range(NCH)}

    with tile.TileContext(nc) as tc, ExitStack() as ctx:
        const = ctx.enter_context(tc.tile_pool(name="const", bufs=1))
        xbp = ctx.enter_context(tc.tile_pool(name="xbp", bufs=BUFS["xbp"]))
        xtp = ctx.enter_context(tc.tile_pool(name="xtp", bufs=BUFS["xtp"]))
        ttp = ctx.enter_context(tc.tile_pool(name="ttp", bufs=BUFS["ttp"]))
        ep = ctx.enter_context(tc.tile_pool(name="ep", bufs=2))
        oep = ctx.enter_context(tc.tile_pool(name="oep", bufs=BUFS["oep"]))
        outp = ctx.enter_context(tc.tile_pool(name="outp", bufs=2))
        smallp = ctx.enter_context(tc.tile_pool(name="smallp", bufs=4))
        ps_xt = ctx.enter_context(tc.tile_pool(name="ps_xt", bufs=2, space="PSUM"))
        ps_u = ctx.enter_context(tc.tile_pool(name="ps_u", bufs=2, space="PSUM"))
        ps_s = ctx.enter_context(tc.tile_pool(name="ps_s", bufs=2, space="PSUM"))
        ps_acc = ctx.enter_context(tc.tile_pool(name="ps_acc", bufs=2, space="PSUM"))

        # --- constants, loaded once (bf16 casts made on-chip) ---
        brel_s = const.tile([128, T], f32)
        nc.sync.dma_start(brel_s[:], brel_d[:])
        b1_s = const.tile([128, 1], f32)
        nc.sync.dma_start(b1_s[:], b1_d[:])
        b2_s = const.tile([128, 1], f32)
        nc.sync.dma_start(b2_s[:], b2_d[:])
        iota_s = const.tile([128, B_LOC], f32)
        nc.sync.dma_start(iota_s[:], iota_d[:])
        w1f_s = const.tile([128, 256], f32)
        nc.sync.dma_start(w1f_s[:, 0:128], w1_d[0])
        nc.sync.dma_start(w1f_s[:, 128:256], w1_d[1])
        w1_s = const.tile([128, 256], bf16)
        nc.vector.tensor_copy(w1_s[:], w1f_s[:])
        w2f_s = const.tile([128, 1], f32)
        nc.sync.dma_start(w2f_s[:], w2_d[:])
        w2_s = const.tile([128, 1], bf16)
        nc.vector.tensor_copy(w2_s[:], w2f_s[:])
        idf_s = const.tile([128, 128], f32)
        nc.sync.dma_start(idf_s[:], id_d[:])
        id_s = const.tile([128, 128], bf16)
        nc.vector.tensor_copy(id_s[:], idf_s[:])

        score_on = ablate not in ("noscore", "dmaonly")
        trans_on = score_on and ablate != "notrans"
        pool_on = ablate not in ("nopool", "dmaonly")
        if not trans_on:
            xdum_s = const.tile([128, 2 * STT * 128], bf16)
            nc.vector.memset(xdum_s[:], 0.01)

        def body(_iv=None):
            acc = {}
            for t0 in range(0, T, CHT):
                # x arrives from HBM already in bf16 (host-side cast)
                xcb = xbp.tile([128, CHT * DA], bf16, tag="xcb")
                if xlayout == "sb":
                    nc.sync.dma_start(xcb[:, :], x_d[:, t0 * DA : (t0 + CHT) * DA])
                else:
                    nc.sync.dma_start(
                        xcb[:, :].rearrange("p (j d) -> p j d", d=DA),
                        x_d[t0 * 128 : (t0 + CHT) * 128, :].rearrange(
                            "(j p) d -> p j d", p=128
                        ),
                    )
                nt = min(CHT, max(0, nt_real - t0))  # tiles with real nodes
                ci = t0 // CHT
                on_pe = pe_every > 0 and ci % pe_every == 0
                if score_on and trans_on and not on_pe and nt > 0:
                    # whole-chunk load of the host-side pre-transposed x copy
                    xtc = xtp.tile([128, 2 * CHT * 128], bf16, tag="xtc")
                    xtq_eng = nc.scalar if xtq == "scalar" else nc.sync
                    for c in (0, 1):
                        xtq_eng.dma_start(
                            xtc[:, c * CHT * 128 : (c + 1) * CHT * 128],
                            xt_d[c, :, t0 * 128 : (t0 + CHT) * 128],
                        )
                s_ps = ps_s.tile([128, CHT], f32, tag="sps")
                for st in range(t0, t0 + nt, STT):
                    if not score_on:
                        break
                    # --- transposed x for the MLP: either 8 PE transposes
                    # (runs at the cold 1.2 GHz transpose clock) or 2 XBAR
                    # DMA-transposes straight from HBM (re-reads x there) ---
                    if trans_on and on_pe:
                        xt_s = xtp.tile([128, 2 * STT * 128], bf16, tag="xts")
                        xt_sv = xt_s[:, :].rearrange("p (c q) -> p c q", c=2)
                        for h in (0, 1):
                            xt_ps = ps_xt.tile([128, 512], bf16, tag="xtps")
                            for jj in (0, 1):
                                j = (st - t0) + 2 * h + jj
                                for c in (0, 1):
                                    nc.tensor.transpose(
                                        xt_ps[:, c * 256 + jj * 128 : c * 256 + jj * 128 + 128],
                                        xcb[:, j * DA + c * 128 : j * DA + c * 128 + 128],
                                        id_s[:],
                                    )
                            src = xt_ps[:, :].rearrange("p (c r) -> p c r", c=2)
                            dst = xt_sv[:, :, h * 256 : h * 256 + 256]
                            if h == 0:
                                nc.scalar.copy(dst, src)
                            else:
                                nc.vector.tensor_copy(dst, src)
                    elif trans_on:
                        soff = (st - t0) * 128
                        xt_s = None
                    else:
                        xt_s = xdum_s
                    if xt_s is not None:
                        xt_lo = xt_s[:, 0:512]
                        xt_hi = xt_s[:, 512:1024]
                    else:
                        xt_lo = xtc[:, soff : soff + 512]
                        xt_hi = xtc[:, CHT * 128 + soff : CHT * 128 + soff + 512]
                    # --- MLP: U^T[h, n] over the 512-node supertile ---
                    u_ps = ps_u.tile([128, 512], f32, tag="ups")
                    nc.tensor.matmul(
                        u_ps[:], w1_s[:, 0:128], xt_lo, start=True, stop=False
                    )
                    nc.tensor.matmul(
                        u_ps[:], w1_s[:, 128:256], xt_hi, start=False, stop=True,
                    )
                    tt_s = ttp.tile([128, 512], bf16, tag="tts")
                    nc.scalar.activation(tt_s[:], u_ps[:], AFT.Tanh, bias=b1_s[:])
                    for j in range(STT):
                        col = (st - t0) + j
                        nc.tensor.matmul(
                            s_ps[:, col : col + 1],
                            tt_s[:, j * 128 : (j + 1) * 128],
                            w2_s[:],
                            start=True,
                            stop=True,
                        )
                # --- e = exp(s + b2) for the whole chunk ---
                e_s = ep.tile([128, CHT], f32, tag="es")
                if score_on and nt > 0:
                    if expsplit and nt > STT:
                        # release the first supertile's pooling before the
                        # rest of the chunk's scores are done
                        nc.scalar.activation(
                            e_s[:, 0:STT], s_ps[:, 0:STT], AFT.Exp, bias=b2_s[:]
                        )
                        nc.scalar.activation(
                            e_s[:, STT:nt], s_ps[:, STT:nt], AFT.Exp, bias=b2_s[:]
                        )
                    else:
                        nc.scalar.activation(
                            e_s[:, 0:nt], s_ps[:, 0:nt], AFT.Exp, bias=b2_s[:]
                        )
                elif not score_on:
                    nc.vector.memset(e_s[:], 1.0)
                # --- pooling matmuls ---
                for j in range(CHT):
                    t = t0 + j
                    xb_t = xcb[:, j * DA : j * DA + DA]
                    for c in range(NCH):
                        if not pool_on:
                            continue
                        if not (first[c] <= t <= last[c]):
                            continue
                        if t == first[c]:
                            acc_t = ps_acc.tile([GCH, DA], f32, tag="acc")
                            acc[c] = acc_t
                        oe_s = oep.tile([128, GCH], bf16, tag="oes")
                        nc.vector.tensor_scalar(
                            oe_s[:],
                            iota_s[:, c * GCH : (c + 1) * GCH],
                            brel_s[:, t : t + 1],
                            e_s[:, j : j + 1],
                            op0=ALU.is_equal,
                            op1=ALU.mult,
                        )
                        nc.tensor.matmul(
                            acc[c][:, 0:DA],
                            oe_s[:],
                            xb_t[:],
                            start=(t == first[c]),
                            stop=(t == last[c]),
                        )
                        if t == last[c]:
                            den = smallp.tile([GCH, 1], f32, tag="den")
                            nc.vector.tensor_scalar_add(
                                den[:], acc[c][:, D : D + 1], 1e-30
                            )
                            rec = smallp.tile([GCH, 1], f32, tag="rec")
                            nc.vector.reciprocal(rec[:], den[:])
                            o_s = outp.tile([GCH, D], f32, tag="os")
                            nc.vector.tensor_scalar_mul(o_s[:], acc[c][:, 0:D], rec[:])
                            nc.sync.dma_start(out_d[c * GCH : (c + 1) * GCH, :], o_s[:])

        if repeat == 1:
            body()
        else:
            with tc.For_i(0, repeat, 1) as _i:
                body(_i)
    nc.compile()
    return nc


def _get_program(C, bnds, nt_real, repeat=1, ablate="", pe_every=None,
                 xlayout=None, expsplit=None, xtq=None):
    if pe_every is None:
        pe_every = PE_EVERY
    if xlayout is None:
        xlayout = XLAYOUT
    if expsplit is None:
        expsplit = EXPSPLIT
    if xtq is None:
        xtq = XTQ
    key = (C, bnds, nt_real, repeat, ablate, pe_every, xlayout, expsplit, xtq)
    if key not in _prog_cache:
        _prog_cache[key] = _build_program(
            C, bnds, nt_real, repeat, ablate, pe_every, xlayout, expsplit, xtq
        )
    return _prog_cache[key]


def _prep_inputs(x, batch, W1, b1, W2, b2, xlayout=None):
    """Host-side sharding: split nodes at graph boundaries, pad to fixed C."""
    if xlayout is None:
        xlayout = XLAYOUT
    x = np.ascontiguousarray(x, dtype=np.float32)
    batch = np.asarray(batch)
    W1 = np.ascontiguousarray(W1, dtype=np.float32)

    bounds = np.searchsorted(batch, np.arange(0, B_FULL + 1, B_LOC))  # [9]
    n_k = np.diff(bounds)
    cap = int(n_k.max())
    gran = 128 * CHT
    C = max(gran, ((cap + gran - 1) // gran) * gran)
    T = C // 128
    nt_real = (cap + 127) // 128

    # graph-chunk tile ranges, shared across cores (min/max over cores)
    bnds = []
    for c in range(NCH):
        los, his = [], []
        for k in range(NCORES):
            g0 = k * B_LOC + c * GCH
            g1 = g0 + GCH
            lo = int(np.searchsorted(batch, g0)) - int(bounds[k])
            hi = int(np.searchsorted(batch, g1)) - int(bounds[k])
            los.append(lo // 128)
            his.append((hi - 1) // 128 if hi > 0 else 0)
        ft = max(0, min(los))
        lt = min(T - 1, max(his))
        if c == NCH - 1:
            lt = nt_real - 1  # padding rows never match any graph
        bnds.append((ft, lt))
    bnds = tuple(bnds)

    shared = {
        "w1": W1.reshape(2, 128, H),
        "b1": np.ascontiguousarray(b1, dtype=np.float32).reshape(H, 1),
        "w2": np.ascontiguousarray(W2, dtype=np.float32).reshape(H, 1),
        "b2": np.full((128, 1), float(np.asarray(b2).reshape(-1)[0]), np.float32),
        "ident": np.eye(128, dtype=np.float32),
        "iota": np.broadcast_to(
            np.arange(B_LOC, dtype=np.float32), (128, B_LOC)
        ).copy(),
    }
    import ml_dtypes

    in_maps = []
    for k in range(NCORES):
        s, e = int(bounds[k]), int(bounds[k + 1])
        n = e - s
        # x ships to HBM in bf16: halves DMA traffic and feeds the all-bf16
        # PE path directly (PSUM accumulation stays fp32)
        xk = np.zeros((C, DA), ml_dtypes.bfloat16)
        xk[:n, :D] = x[s:e].astype(ml_dtypes.bfloat16)
        xk[:, D] = 1.0  # denominator ones column (pad rows masked by one-hot)
        # pre-transposed copy for the MLP path (layout prep, not compute)
        xtk = np.ascontiguousarray(xk[:, :D].T).reshape(2, 128, C)
        if xlayout == "sb":
            # exact SBUF chunk layout: [128 partitions, (tile, col)]
            xk = np.ascontiguousarray(
                xk.reshape(T, 128, DA).transpose(1, 0, 2).reshape(128, T * DA)
            )
        br = np.full((C,), PAD_SENTINEL, np.float32)
        br[:n] = batch[s:e].astype(np.float32) - k * B_LOC
        in_maps.append(
            {"x": xk, "xt": xtk,
             "brel": np.ascontiguousarray(br.reshape(T, 128).T), **shared}
        )
    return in_maps, C, bnds, nt_real


def kernel(x, batch, W1, b1, W2, b2):
    from concourse.bass_utils import run_bass_kernel_spmd

    in_maps, C, bnds, nt_real = _prep_inputs(x, batch, W1, b1, W2, b2)
    nc = _get_program(C, bnds, nt_real)
    res = run_bass_kernel_spmd(nc, in_maps, list(range(NCORES)))
    out = np.concatenate([res.results[k]["out"] for k in range(NCORES)], axis=0)
    return np.ascontiguousarray(out, dtype=np.float32)



# revision 9
# speedup vs baseline: 1.1274x; 1.1274x over previous
"""Trainium2 Bass kernel for ContractLevelAttention (segment softmax-pooling).

Computes, for x:[N,D], sorted batch:[N] (graph ids in [0,B)), MLP weights:
    scores = tanh(x @ W1 + b1) @ W2 + b2              # [N]
    w      = segment_softmax(scores, batch)           # per-graph softmax
    out    = segment_sum(x * w[:, None], batch)       # [B, D]

Key facts exploited:
  * softmax is shift invariant and |scores| <= 1 + 128*max|W2| + |b2| ~ 11.5
    (tanh output bounded), so exp() never overflows in fp32 and the
    segment-max subtraction of the reference can be dropped entirely.
  * out[g] = (sum_i e_i x_i) / (sum_i e_i) over i in graph g, so the
    normalization happens once at the end -- both sums are plain
    segment-sums, done as one-hot matmuls on the PE.
  * the whole PE path runs in bf16 (1 cyc/row vs 4 for fp32): the host ships
    x already cast to bf16 (halves HBM traffic) and in the exact SBUF chunk
    layout; PSUM accumulation stays fp32 so segment sums are fp32-accurate.
  * PE transpose-mode runs at the cold 1.2 GHz clock, so most chunks instead
    load a host-side pre-transposed copy of x for the MLP (pure layout prep);
    1 in PE_EVERY chunks still transposes on the PE to balance DMA vs PE.

Sharding: graph-level data parallel over 8 cores (batch is sorted, so each
core's nodes are one contiguous slice, zero-padded to a fixed capacity).
"""

import numpy as np
from contextlib import ExitStack

N_FULL = 524288
D = 256
H = 128
B_FULL = 2048
NCORES = 8
B_LOC = B_FULL // NCORES      # 256 graphs per core
GCH = 128                     # graphs per PSUM accumulator chunk
NCH = B_LOC // GCH            # accumulator chunks per core
PAD_SENTINEL = 3.0 * B_LOC    # batch_rel value for padding rows (never matches)
CHT = 16                      # 128-node tiles per x DMA chunk (~2MB loads)
STT = 4                       # tiles per compute supertile
DA = D + 2                    # x cols + ones col (denominator) + pad col

_prog_cache = {}


PE_EVERY = 0  # 1 of every PE_EVERY chunks transposes on the PE; the rest
              # load the host-side pre-transposed x copy (0 = all DMA)
XLAYOUT = "sb"  # 'sb' = x shipped in SBUF chunk layout; 'row' = row-major
BUFS = {"xbp": 3, "xtp": 3, "ttp": 3, "oep": 4}
EXPSPLIT = False  # split the per-chunk exp so pooling starts earlier
XTQ = "sync"      # HWDGE queue for the xT loads: 'sync' or 'scalar'
XTF8 = True       # ship the transposed x copy in fp8 e4m3: halves that DMA
                  # stream and runs the MLP matmul in DoubleRow mode (2 rows
                  # of the d-contraction per PE cycle). Softmax weights are
                  # insensitive to score quantization (~1e-2 rel out err).


def _build_program(C, bnds, nt_real, repeat=1, ablate="", pe_every=None,
                   xlayout=None, expsplit=None, xtq=None):
    """Build the per-core SPMD program. C = padded node capacity (multiple of
    128*CHT). bnds = tuple of (first_tile, last_tile) per graph chunk, shared
    across cores (min/max over cores). nt_real = number of tiles containing
    any real (non-padding) node. repeat>1 wraps the body in an on-device
    loop (for timing)."""
    import concourse.bass as bass
    from concourse import bacc, mybir
    import concourse.tile as tile

    if pe_every is None:
        pe_every = PE_EVERY
    if xlayout is None:
        xlayout = XLAYOUT
    if expsplit is None:
        expsplit = EXPSPLIT
    if xtq is None:
        xtq = XTQ
    f32 = mybir.dt.float32
    bf16 = mybir.dt.bfloat16
    f8 = mybir.dt.float8e4
    xt_dt = f8 if XTF8 else bf16
    AFT = mybir.ActivationFunctionType
    ALU = mybir.AluOpType
    T = C // 128

    nc = bacc.Bacc(
        "TRN2",
        target_bir_lowering=False,
        debug=False,
        enable_asserts=False,
        num_devices=NCORES,
    )
    # 'sb': x shipped in the exact SBUF chunk layout [128, T*DA] (host prep):
    # partition p, col block (t, d) holds x[t*128 + p, d] -> fully
    # contiguous 8.25KB-per-partition DMA descriptors per chunk.
    if xlayout == "sb":
        x_d = nc.dram_tensor("x", [128, T * DA], bf16, kind="ExternalInput").ap()
    else:
        x_d = nc.dram_tensor("x", [C, DA], bf16, kind="ExternalInput").ap()
    xt_d = nc.dram_tensor("xt", [2, 128, C], xt_dt, kind="ExternalInput").ap()
    brel_d = nc.dram_tensor("brel", [128, T], f32, kind="ExternalInput").ap()
    w1_d = nc.dram_tensor("w1", [2, 128, H], f32, kind="ExternalInput").ap()
    b1_d = nc.dram_tensor("b1", [H, 1], f32, kind="ExternalInput").ap()
    w2_d = nc.dram_tensor("w2", [H, 1], f32, kind="ExternalInput").ap()
    b2_d = nc.dram_tensor("b2", [128, 1], f32, kind="ExternalInput").ap()
    id_d = nc.dram_tensor("ident", [128, 128], f32, kind="ExternalInput").ap()
    iota_d = nc.dram_tensor("iota", [128, B_LOC], f32, kind="ExternalInput").ap()
    out_d = nc.dram_tensor("out", [B_LOC, D], f32, kind="ExternalOutput").ap()

    first = {c: bnds[c][0] for c in range(NCH)}
    last = {c: bnds[c][1] for c in range(NCH)}

    with tile.TileContext(nc) as tc, ExitStack() as ctx:
        const = ctx.enter_context(tc.tile_pool(name="const", bufs=1))
        xbp = ctx.enter_context(tc.tile_pool(name="xbp", bufs=BUFS["xbp"]))
        xtp = ctx.enter_context(tc.tile_pool(name="xtp", bufs=BUFS["xtp"]))
        ttp = ctx.enter_context(tc.tile_pool(name="ttp", bufs=BUFS["ttp"]))
        ep = ctx.enter_context(tc.tile_pool(name="ep", bufs=2))
        oep = ctx.enter_context(tc.tile_pool(name="oep", bufs=BUFS["oep"]))
        outp = ctx.enter_context(tc.tile_pool(name="outp", bufs=2))
        smallp = ctx.enter_context(tc.tile_pool(name="smallp", bufs=4))
        ps_xt = ctx.enter_context(tc.tile_pool(name="ps_xt", bufs=2, space="PSUM"))
        ps_u = ctx.enter_context(tc.tile_pool(name="ps_u", bufs=2, space="PSUM"))
        ps_s = ctx.enter_context(tc.tile_pool(name="ps_s", bufs=2, space="PSUM"))
        ps_acc = ctx.enter_context(tc.tile_pool(name="ps_acc", bufs=2, space="PSUM"))

        # --- constants, loaded once (bf16 casts made on-chip) ---
        brel_s = const.tile([128, T], f32)
        nc.sync.dma_start(brel_s[:], brel_d[:])
        b1_s = const.tile([128, 1], f32)
        nc.sync.dma_start(b1_s[:], b1_d[:])
        b2_s = const.tile([128, 1], f32)
        nc.sync.dma_start(b2_s[:], b2_d[:])
        iota_s = const.tile([128, B_LOC], f32)
        nc.sync.dma_start(iota_s[:], iota_d[:])
        w1f_s = const.tile([128, 256], f32)
        nc.sync.dma_start(w1f_s[:, 0:128], w1_d[0])
        nc.sync.dma_start(w1f_s[:, 128:256], w1_d[1])
        w1_s = const.tile([128, 256], bf16)
        nc.vector.tensor_copy(w1_s[:], w1f_s[:])
        w1_8 = const.tile([128, 2, 128], f8)
        nc.vector.tensor_copy(
            w1_8[:, :, :], w1f_s[:, :].rearrange("p (b h) -> p b h", b=2)
        )
        w2f_s = const.tile([128, 1], f32)
        nc.sync.dma_start(w2f_s[:], w2_d[:])
        w2_s = const.tile([128, 1], bf16)
        nc.vector.tensor_copy(w2_s[:], w2f_s[:])
        idf_s = const.tile([128, 128], f32)
        nc.sync.dma_start(idf_s[:], id_d[:])
        id_s = const.tile([128, 128], bf16)
        nc.vector.tensor_copy(id_s[:], idf_s[:])

        score_on = ablate not in ("noscore", "dmaonly")
        trans_on = score_on and ablate != "notrans"
        pool_on = ablate not in ("nopool", "dmaonly")
        if not trans_on:
            xdum_s = const.tile([128, 2 * STT * 128], bf16)
            nc.vector.memset(xdum_s[:], 0.01)

        def body(_iv=None):
            acc = {}
            for t0 in range(0, T, CHT):
                # x arrives from HBM already in bf16 (host-side cast)
                xcb = xbp.tile([128, CHT * DA], bf16, tag="xcb")
                if xlayout == "sb":
                    nc.sync.dma_start(xcb[:, :], x_d[:, t0 * DA : (t0 + CHT) * DA])
                else:
                    nc.sync.dma_start(
                        xcb[:, :].rearrange("p (j d) -> p j d", d=DA),
                        x_d[t0 * 128 : (t0 + CHT) * 128, :].rearrange(
                            "(j p) d -> p j d", p=128
                        ),
                    )
                nt = min(CHT, max(0, nt_real - t0))  # tiles with real nodes
                ci = t0 // CHT
                on_pe = pe_every > 0 and ci % pe_every == 0
                if score_on and trans_on and not on_pe and nt > 0:
                    # whole-chunk load of the host-side pre-transposed x copy
                    xtc = xtp.tile([128, 2, CHT * 128], xt_dt, tag="xtc")
                    xtq_eng = nc.scalar if xtq == "scalar" else nc.sync
                    for c in (0, 1):
                        xtq_eng.dma_start(
                            xtc[:, c, :],
                            xt_d[c, :, t0 * 128 : (t0 + CHT) * 128],
                        )
                s_ps = ps_s.tile([128, CHT], f32, tag="sps")
                for st in range(t0, t0 + nt, STT):
                    if not score_on:
                        break
                    # --- transposed x for the MLP: either 8 PE transposes
                    # (runs at the cold 1.2 GHz transpose clock) or 2 XBAR
                    # DMA-transposes straight from HBM (re-reads x there) ---
                    if trans_on and on_pe:
                        xt_s = xtp.tile([128, 2 * STT * 128], bf16, tag="xts")
                        xt_sv = xt_s[:, :].rearrange("p (c q) -> p c q", c=2)
                        for h in (0, 1):
                            xt_ps = ps_xt.tile([128, 512], bf16, tag="xtps")
                            for jj in (0, 1):
                                j = (st - t0) + 2 * h + jj
                                for c in (0, 1):
                                    nc.tensor.transpose(
                                        xt_ps[:, c * 256 + jj * 128 : c * 256 + jj * 128 + 128],
                                        xcb[:, j * DA + c * 128 : j * DA + c * 128 + 128],
                                        id_s[:],
                                    )
                            src = xt_ps[:, :].rearrange("p (c r) -> p c r", c=2)
                            dst = xt_sv[:, :, h * 256 : h * 256 + 256]
                            if h == 0:
                                nc.scalar.copy(dst, src)
                            else:
                                nc.vector.tensor_copy(dst, src)
                    elif trans_on:
                        soff = (st - t0) * 128
                        xt_s = None
                    else:
                        xt_s = xdum_s
                    # --- MLP: U^T[h, n] over the 512-node supertile ---
                    u_ps = ps_u.tile([128, 512], f32, tag="ups")
                    if xt_s is None and XTF8:
                        # fp8 DoubleRow: both 128-blocks of the d-contraction
                        # in one matmul at 2 rows/cycle (4x fewer PE cycles)
                        nc.tensor.matmul(
                            u_ps[:],
                            w1_8[:, :, :],
                            xtc[:, :, soff : soff + 512],
                            start=True,
                            stop=True,
                            perf_mode=mybir.MatmulPerfMode.DoubleRow,
                        )
                    else:
                        if xt_s is not None:
                            xt_lo = xt_s[:, 0:512]
                            xt_hi = xt_s[:, 512:1024]
                        else:
                            xt_lo = xtc[:, 0, soff : soff + 512]
                            xt_hi = xtc[:, 1, soff : soff + 512]
                        nc.tensor.matmul(
                            u_ps[:], w1_s[:, 0:128], xt_lo, start=True, stop=False
                        )
                        nc.tensor.matmul(
                            u_ps[:], w1_s[:, 128:256], xt_hi, start=False, stop=True,
                        )
                    tt_s = ttp.tile([128, 512], bf16, tag="tts")
                    nc.scalar.activation(tt_s[:], u_ps[:], AFT.Tanh, bias=b1_s[:])
                    for j in range(STT):
                        col = (st - t0) + j
                        nc.tensor.matmul(
                            s_ps[:, col : col + 1],
                            tt_s[:, j * 128 : (j + 1) * 128],
                            w2_s[:],
                            start=True,
                            stop=True,
                        )
                # --- e = exp(s + b2) for the whole chunk ---
                e_s = ep.tile([128, CHT], f32, tag="es")
                if score_on and nt > 0:
                    if expsplit and nt > STT:
                        # release the first supertile's pooling before the
                        # rest of the chunk's scores are done
                        nc.scalar.activation(
                            e_s[:, 0:STT], s_ps[:, 0:STT], AFT.Exp, bias=b2_s[:]
                        )
                        nc.scalar.activation(
                            e_s[:, STT:nt], s_ps[:, STT:nt], AFT.Exp, bias=b2_s[:]
                        )
                    else:
                        nc.scalar.activation(
                            e_s[:, 0:nt], s_ps[:, 0:nt], AFT.Exp, bias=b2_s[:]
                        )
                elif not score_on:
                    nc.vector.memset(e_s[:], 1.0)
                # --- pooling matmuls ---
                for j in range(CHT):
                    t = t0 + j
                    xb_t = xcb[:, j * DA : j * DA + DA]
                    for c in range(NCH):
                        if not pool_on:
                            continue
                        if not (first[c] <= t <= last[c]):
                            continue
                        if t == first[c]:
                            acc_t = ps_acc.tile([GCH, DA], f32, tag="acc")
                            acc[c] = acc_t
                        oe_s = oep.tile([128, GCH], bf16, tag="oes")
                        nc.vector.tensor_scalar(
                            oe_s[:],
                            iota_s[:, c * GCH : (c + 1) * GCH],
                            brel_s[:, t : t + 1],
                            e_s[:, j : j + 1],
                            op0=ALU.is_equal,
                            op1=ALU.mult,
                        )
                        nc.tensor.matmul(
                            acc[c][:, 0:DA],
                            oe_s[:],
                            xb_t[:],
                            start=(t == first[c]),
                            stop=(t == last[c]),
                        )
                        if t == last[c]:
                            den = smallp.tile([GCH, 1], f32, tag="den")
                            nc.vector.tensor_scalar_add(
                                den[:], acc[c][:, D : D + 1], 1e-30
                            )
                            rec = smallp.tile([GCH, 1], f32, tag="rec")
                            nc.vector.reciprocal(rec[:], den[:])
                            o_s = outp.tile([GCH, D], f32, tag="os")
                            nc.vector.tensor_scalar_mul(o_s[:], acc[c][:, 0:D], rec[:])
                            nc.sync.dma_start(out_d[c * GCH : (c + 1) * GCH, :], o_s[:])

        if repeat == 1:
            body()
        else:
            with tc.For_i(0, repeat, 1) as _i:
                body(_i)
    nc.compile()
    return nc


def _get_program(C, bnds, nt_real, repeat=1, ablate="", pe_every=None,
                 xlayout=None, expsplit=None, xtq=None):
    if pe_every is None:
        pe_every = PE_EVERY
    if xlayout is None:
        xlayout = XLAYOUT
    if expsplit is None:
        expsplit = EXPSPLIT
    if xtq is None:
        xtq = XTQ
    key = (C, bnds, nt_real, repeat, ablate, pe_every, xlayout, expsplit, xtq)
    if key not in _prog_cache:
        _prog_cache[key] = _build_program(
            C, bnds, nt_real, repeat, ablate, pe_every, xlayout, expsplit, xtq
        )
    return _prog_cache[key]


def _prep_inputs(x, batch, W1, b1, W2, b2, xlayout=None):
    """Host-side sharding: split nodes at graph boundaries, pad to fixed C."""
    if xlayout is None:
        xlayout = XLAYOUT
    x = np.ascontiguousarray(x, dtype=np.float32)
    batch = np.asarray(batch)
    W1 = np.ascontiguousarray(W1, dtype=np.float32)

    bounds = np.searchsorted(batch, np.arange(0, B_FULL + 1, B_LOC))  # [9]
    n_k = np.diff(bounds)
    cap = int(n_k.max())
    gran = 128 * CHT
    C = max(gran, ((cap + gran - 1) // gran) * gran)
    T = C // 128
    nt_real = (cap + 127) // 128

    # graph-chunk tile ranges, shared across cores (min/max over cores)
    bnds = []
    for c in range(NCH):
        los, his = [], []
        for k in range(NCORES):
            g0 = k * B_LOC + c * GCH
            g1 = g0 + GCH
            lo = int(np.searchsorted(batch, g0)) - int(bounds[k])
            hi = int(np.searchsorted(batch, g1)) - int(bounds[k])
            los.append(lo // 128)
            his.append((hi - 1) // 128 if hi > 0 else 0)
        ft = max(0, min(los))
        lt = min(T - 1, max(his))
        if c == NCH - 1:
            lt = nt_real - 1  # padding rows never match any graph
        bnds.append((ft, lt))
    bnds = tuple(bnds)

    shared = {
        "w1": W1.reshape(2, 128, H),
        "b1": np.ascontiguousarray(b1, dtype=np.float32).reshape(H, 1),
        "w2": np.ascontiguousarray(W2, dtype=np.float32).reshape(H, 1),
        "b2": np.full((128, 1), float(np.asarray(b2).reshape(-1)[0]), np.float32),
        "ident": np.eye(128, dtype=np.float32),
        "iota": np.broadcast_to(
            np.arange(B_LOC, dtype=np.float32), (128, B_LOC)
        ).copy(),
    }
    import ml_dtypes

    in_maps = []
    for k in range(NCORES):
        s, e = int(bounds[k]), int(bounds[k + 1])
        n = e - s
        # x ships to HBM in bf16: halves DMA traffic and feeds the all-bf16
        # PE path directly (PSUM accumulation stays fp32)
        xk = np.zeros((C, DA), ml_dtypes.bfloat16)
        xk[:n, :D] = x[s:e].astype(ml_dtypes.bfloat16)
        xk[:, D] = 1.0  # denominator ones column (pad rows masked by one-hot)
        # pre-transposed copy for the MLP path (layout prep, not compute);
        # fp8 e4m3 when XTF8 (softmax scores tolerate the quantization)
        xt_np = ml_dtypes.float8_e4m3 if XTF8 else ml_dtypes.bfloat16
        xtk = np.zeros((D, C), xt_np)
        xtk[:, :n] = x[s:e].T.astype(xt_np)
        xtk = xtk.reshape(2, 128, C)
        if xlayout == "sb":
            # exact SBUF chunk layout: [128 partitions, (tile, col)]
            xk = np.ascontiguousarray(
                xk.reshape(T, 128, DA).transpose(1, 0, 2).reshape(128, T * DA)
            )
        br = np.full((C,), PAD_SENTINEL, np.float32)
        br[:n] = batch[s:e].astype(np.float32) - k * B_LOC
        in_maps.append(
            {"x": xk, "xt": xtk,
             "brel": np.ascontiguousarray(br.reshape(T, 128).T), **shared}
        )
    return in_maps, C, bnds, nt_real


def kernel(x, batch, W1, b1, W2, b2):
    from concourse.bass_utils import run_bass_kernel_spmd

    in_maps, C, bnds, nt_real = _prep_inputs(x, batch, W1, b1, W2, b2)
    nc = _get_program(C, bnds, nt_real)
    res = run_bass_kernel_spmd(nc, in_maps, list(range(NCORES)))
    out = np.concatenate([res.results[k]["out"] for k in range(NCORES)], axis=0)
    return np.ascontiguousarray(out, dtype=np.float32)

